# revision 1
# baseline (speedup 1.0000x reference)
"""BERT-base (12-layer, B=8, T=512, D=768) forward + tied-embedding LM head
on 8 Trainium2 NeuronCores.

Sharding: data-parallel over the batch dimension -- core b computes batch
element b end-to-end (no collectives). Activations are kept feature-major
[D, T] in SBUF so every GEMM consumes weights in their natural [d_in, d_out]
layout; attention scores are computed key-major so softmax reductions become
ones-matmuls / an appended ones-column on V; the LM head runs token-major so
logits come out [T, V] without any transposes. All GEMMs run in bf16 with
fp32 PSUM accumulation; the residual stream stays fp32.
"""

from contextlib import ExitStack

import numpy as np
import ml_dtypes

import concourse.bass as bass
import concourse.bacc as bacc
import concourse.mybir as mybir
import concourse.tile as tile
from concourse import bass_utils
from concourse._compat import get_trn_type

NP_BF16 = ml_dtypes.bfloat16

FP32 = mybir.dt.float32
BF16 = mybir.dt.bfloat16
AF = mybir.ActivationFunctionType
OP = mybir.AluOpType

P = 128
T = 512
D = 768
H = 12
HD = 64
DF = 3072
V = 30522
DK = D // P      # 6 contraction chunks over D
TCH = T // P     # 4 token chunks
FK = DF // P     # 24 contraction chunks over DF
SCALE = 0.125    # 1/sqrt(64)
EPS = 1e-5


def bcast_ap(t, nparts, free):
    """partition-broadcast view of a [1, free] sbuf tile -> [nparts, free]"""
    return bass.AP(tensor=t.tensor, offset=t.offset, ap=[[0, nparts], [1, free]])


def build(n_layers=12, with_head=True, debug_taps=()):
    nc = bacc.Bacc(get_trn_type() or "TRN2", target_bir_lowering=False, debug=False)

    x0T = nc.dram_tensor("x0T", [D, T], FP32, kind="ExternalInput")
    mb = nc.dram_tensor("mb", [P, TCH], FP32, kind="ExternalInput")
    L = max(n_layers, 1)
    wqk = nc.dram_tensor("wqk", [L, D, 2 * D], BF16, kind="ExternalInput")
    wv = nc.dram_tensor("wv", [L, D, D], BF16, kind="ExternalInput")
    wpr = nc.dram_tensor("wpr", [L, D, D], BF16, kind="ExternalInput")
    wfc = nc.dram_tensor("wfc", [L, D, DF], BF16, kind="ExternalInput")
    wf2 = nc.dram_tensor("wf2", [L, DF, D], BF16, kind="ExternalInput")
    if with_head:
        wembT = nc.dram_tensor("wembT", [D, V], BF16, kind="ExternalInput")
        out = nc.dram_tensor("out", [T, V], FP32, kind="ExternalOutput")
    else:
        out = nc.dram_tensor("out", [D, T], FP32, kind="ExternalOutput")

    tap_specs = {
        "h1": ([D, T], BF16), "qkT": ([2 * D, T], BF16),
        "v65": ([TCH * P, H * (HD + 1)], BF16), "p0": ([TCH * P, T], BF16),
        "yT": ([D, T], BF16), "xattn": ([D, T], FP32), "h2": ([D, T], BF16),
        "gT": ([DF, T], BF16), "xfinal": ([D, T], FP32),
        "yraw": ([H * (HD + 1), T], FP32), "invb": ([H, T], FP32),
    }
    taps = {}
    for name in debug_taps:
        shape, dt = tap_specs[name]
        taps[name] = nc.dram_tensor(f"tap_{name}", shape, dt, kind="ExternalOutput")

    with tile.TileContext(nc) as tc, ExitStack() as ctx:
        consts = ctx.enter_context(tc.tile_pool(name="consts", bufs=1))
        resid = ctx.enter_context(tc.tile_pool(name="resid", bufs=1))
        acts = ctx.enter_context(tc.tile_pool(name="acts", bufs=2))
        wpool = ctx.enter_context(tc.tile_pool(name="wpool", bufs=1))
        small = ctx.enter_context(tc.tile_pool(name="small", bufs=1))
        ps_stat = ctx.enter_context(tc.tile_pool(name="ps_stat", bufs=2, space="PSUM"))
        ps_gemm = ctx.enter_context(tc.tile_pool(name="ps_gemm", bufs=4, space="PSUM"))
        ps_av = ctx.enter_context(tc.tile_pool(name="ps_av", bufs=2, space="PSUM"))
        dscratch = ctx.enter_context(tc.tile_pool(name="dscratch", bufs=4, space="DRAM"))

        czero = consts.tile([P, 1], FP32, tag="czero")
        nc.vector.memset(czero[:], 0.0)
        ceps = consts.tile([P, 1], FP32, tag="ceps")
        nc.vector.memset(ceps[:], EPS)
        nc.const_aps.aps[(FP32, 0.0)] = czero[:]
        nc.const_aps.aps[(FP32, EPS)] = ceps[:]

        ones_f32 = consts.tile([P, 1], FP32, tag="ones_f32")
        nc.vector.memset(ones_f32[:], 1.0)
        ones_bf = consts.tile([P, 1], BF16, tag="ones_bf")
        nc.vector.memset(ones_bf[:], 1.0)
        ones_row = consts.tile([1, P], FP32, tag="ones_row")
        nc.vector.memset(ones_row[:], 1.0)
        mb_sb = consts.tile([P, TCH], FP32, tag="mb_sb")
        nc.sync.dma_start(mb_sb[:], mb[:])

        # residual stream
        xT = []
        for j in range(DK):
            t = resid.tile([P, T], FP32, tag=f"x{j}")
            nc.sync.dma_start(t[:], x0T[j * P:(j + 1) * P, :])
            xT.append(t)

        def layer_norm(tag):
            """feature-major LN over xT -> 6 bf16 tiles"""
            sum_ps = ps_stat.tile([1, T], FP32, tag="stat")
            ssq_ps = ps_stat.tile([1, T], FP32, tag="stat")
            sq_tiles = []
            for j in range(DK):
                sq = acts.tile([P, T], BF16, tag="sq", bufs=6)
                nc.scalar.activation(sq[:], xT[j][:], AF.Square)
                sq_tiles.append(sq)
                nc.tensor.matmul(sum_ps[:], ones_f32[:], xT[j][:],
                                 start=(j == 0), stop=(j == DK - 1))
            for j in range(DK):
                nc.tensor.matmul(ssq_ps[:], ones_bf[:], sq_tiles[j][:],
                                 start=(j == 0), stop=(j == DK - 1))
            nm = small.tile([1, T], FP32, tag="nm")
            nc.vector.tensor_scalar_mul(nm[:], sum_ps[:], -1.0 / D)
            msq = small.tile([1, T], FP32, tag="msq")
            nc.vector.tensor_mul(msq[:], nm[:], nm[:])
            var = small.tile([1, T], FP32, tag="var")
            nc.vector.scalar_tensor_tensor(
                out=var[:], in0=ssq_ps[:], scalar=1.0 / D, in1=msq[:],
                op0=OP.mult, op1=OP.subtract)
            lnv = small.tile([1, T], FP32, tag="lnv")
            nc.scalar.activation(lnv[:], var[:], AF.Ln, bias=EPS)
            rstd = small.tile([1, T], FP32, tag="rstd")
            nc.scalar.activation(rstd[:], lnv[:], AF.Exp, scale=-0.5)
            nmrs = small.tile([1, T], FP32, tag="nmrs")  # -mean*rstd
            nc.vector.tensor_mul(nmrs[:], nm[:], rstd[:])
            # broadcast rstd and -mean*rstd across partitions via ones-matmul
            rstd_b = ps_stat.tile([P, T], FP32, tag="stat", name="rstd_b")
            nc.tensor.matmul(rstd_b[:], ones_row[:], rstd[:], start=True, stop=True)
            nmrs_b = ps_stat.tile([P, T], FP32, tag="stat", name="nmrs_b")
            nc.tensor.matmul(nmrs_b[:], ones_row[:], nmrs[:], start=True, stop=True)
            h_tiles = []
            for j in range(DK):
                h = acts.tile([P, T], BF16, tag=f"h_{tag}", bufs=6)
                # h = x*rstd_b + nmrs_b
                nc.vector.tensor_mul(h[:], xT[j][:], rstd_b[:])
                nc.vector.tensor_add(h[:], h[:], nmrs_b[:])
                h_tiles.append(h)
            return h_tiles

        def gemm_fm(w3, l, M, rhs_tiles, tag, CG, evac, nk=DK):
            """feature-major GEMM: for each m-chunk of M, psum[128,T] =
            sum_k w3[l, k*128:(k+1)*128, m-chunk].T @ rhs_tiles[k]; column
            groups of CG limit slab residency."""
            for cg0 in range(0, M, CG):
                cgn = min(CG, M - cg0)
                slabs = []
                for k in range(nk):
                    s = wpool.tile([P, CG], BF16, tag=f"{tag}_{k}", bufs=2)
                    nc.sync.dma_start(s[:, :cgn], w3[l, k * P:(k + 1) * P, cg0:cg0 + cgn])
                    slabs.append(s)
                for mi in range(cgn // P):
                    m = (cg0 // P) + mi
                    ps = ps_gemm.tile([P, T], FP32, tag="g")
                    for k in range(nk):
                        nc.tensor.matmul(ps[:], slabs[k][:, mi * P:(mi + 1) * P],
                                         rhs_tiles[k][:],
                                         start=(k == 0), stop=(k == nk - 1))
                    evac(m, ps)

        def dump_tiles(name, tiles, rows=P):
            if name in taps:
                for j, t in enumerate(tiles):
                    nc.sync.dma_start(taps[name][j * rows:(j + 1) * rows, :], t[:])

        def layer(l):
            h1 = layer_norm("ln1")
            dump_tiles("h1", h1)

            # ---- QK gemm (feature-major): qkT[c,t], c in [0,1536) ----
            qkT = [None] * (2 * D // P)

            def qk_evac(m, ps):
                qt = acts.tile([P, T], BF16, tag="qkT", bufs=12)
                nc.vector.tensor_copy(qt[:], ps[:])
                qkT[m] = qt
            gemm_fm(wqk, l, 2 * D, h1, "wqk", T, qk_evac)
            dump_tiles("qkT", qkT)

            # ---- V gemm (token-major): v[t, c] with ones column per head ----
            v_slabs = []
            for k in range(DK):
                s = wpool.tile([P, D], BF16, tag=f"wv_{k}", bufs=1)
                nc.sync.dma_start(s[:], wv[l, k * P:(k + 1) * P, :])
                v_slabs.append(s)
            v65 = []
            for tch in range(TCH):
                vt = acts.tile([P, H, HD + 1], BF16, tag="v65", bufs=5)
                nc.vector.memset(vt[:, :, HD:HD + 1], 1.0)
                for n0 in range(0, D, T):  # n in {0, 512} sizes {512, 256}
                    nn = min(T, D - n0)
                    ps = ps_gemm.tile([P, T], FP32, tag="g")
                    for k in range(DK):
                        nc.tensor.matmul(
                            ps[:, :nn],
                            h1[k][:, tch * P:(tch + 1) * P],
                            v_slabs[k][:, n0:n0 + nn],
                            start=(k == 0), stop=(k == DK - 1))
                    dst = vt[:, n0 // HD:(n0 + nn) // HD, 0:HD]
                    src = ps[:, :nn].rearrange("p (h d) -> p h d", d=HD)
                    nc.vector.tensor_copy(dst, src)
                v65.append(vt)
            if "v65" in taps:
                for j, t in enumerate(v65):
                    nc.sync.dma_start(
                        taps["v65"][j * P:(j + 1) * P, :],
                        t[:].rearrange("p h d -> p (h d)"))

            # ---- attention per head ----
            yT = [acts.tile([P, T], BF16, tag="yT", bufs=6, name=f"yT{i}")
                  for i in range(DK)]
            for h in range(H):
                ht, r = h // 2, h % 2
                qt = qkT[ht]
                kt = qkT[DK + ht]
                rows = slice(r * HD, (r + 1) * HD)
                p_tiles = []
                for kc in range(TCH):
                    s_ps = ps_gemm.tile([P, T], FP32, tag="g")
                    nc.tensor.matmul(s_ps[:], kt[rows, kc * P:(kc + 1) * P],
                                     qt[rows, :], start=True, stop=True)
                    pt = acts.tile([P, T], BF16, tag="p", bufs=5)
                    nc.scalar.activation(pt[:], s_ps[:], AF.Exp,
                                         bias=mb_sb[:, kc:kc + 1], scale=SCALE)
                    p_tiles.append(pt)
                if h == 0 and "p0" in taps:
                    for kc in range(TCH):
                        nc.sync.dma_start(
                            taps["p0"][kc * P:(kc + 1) * P, :], p_tiles[kc][:])
                y_ps = ps_av.tile([HD + 1, T], FP32, tag="av")
                for kc in range(TCH):
                    nc.tensor.matmul(y_ps[:], v65[kc][:, h, :], p_tiles[kc][:],
                                     start=(kc == 0), stop=(kc == TCH - 1))
                if "yraw" in taps:
                    yr = acts.tile([HD + 1, T], FP32, tag="yraw", bufs=2)
                    nc.vector.tensor_copy(yr[:], y_ps[:])
                    nc.sync.dma_start(
                        taps["yraw"][h * (HD + 1):(h + 1) * (HD + 1), :], yr[:])
                # reciprocal of the sum row: lanes are partition-locked, so
                # stage at partition HD, round-trip through DRAM to broadcast
                # 1/sum = exp(-ln(sum)) on ACT (same table set as softmax exp;
                # reciprocal_approx_fast only works at partition base 0)
                lns = acts.tile([HD + 1, T], BF16, tag="lns", bufs=2)
                nc.scalar.activation(lns[HD:HD + 1, :], y_ps[HD:HD + 1, :], AF.Ln)
                inv_st = acts.tile([HD + 1, T], BF16, tag="inv_st", bufs=2)
                nc.scalar.activation(inv_st[HD:HD + 1, :], lns[HD:HD + 1, :],
                                     AF.Exp, scale=-1.0)
                invd = dscratch.tile([1, T], BF16, tag="invd", bufs=4)
                nc.sync.dma_start(invd[:], inv_st[HD:HD + 1, :])
                inv_b = acts.tile([HD, T], BF16, tag="inv_b", bufs=2)
                nc.sync.dma_start(inv_b[:], bcast_ap(invd, HD, T))
                if "invb" in taps:
                    nc.sync.dma_start(taps["invb"][h:h + 1, :], inv_st[HD:HD + 1, :])
                if r == 0:
                    nc.vector.tensor_mul(yT[ht][rows, :], y_ps[0:HD, :], inv_b[:])
                else:
                    ytmp = acts.tile([HD, T], BF16, tag="ytmp", bufs=2)
                    nc.vector.tensor_mul(ytmp[:], y_ps[0:HD, :], inv_b[:])
                    nc.sync.dma_start(yT[ht][rows, :], ytmp[:])

            dump_tiles("yT", yT)

            # ---- proj gemm + residual ----
            def resid_evac(m, ps):
                nc.vector.tensor_add(xT[m][:], xT[m][:], ps[:])
            gemm_fm(wpr, l, D, yT, "wpr", 3 * P, resid_evac)

            dump_tiles("xattn", xT)
            h2 = layer_norm("ln2")
            dump_tiles("h2", h2)

            # ---- fc1 gemm + gelu ----
            gT = [None] * FK

            def gelu_evac(m, ps):
                g = acts.tile([P, T], BF16, tag="gT", bufs=24)
                nc.scalar.activation(g[:], ps[:], AF.Gelu_apprx_tanh)
                gT[m] = g
            gemm_fm(wfc, l, DF, h2, "wfc", 2 * 3 * P, gelu_evac)

            dump_tiles("gT", gT)

            # ---- fc2 gemm + residual ----
            gemm_fm(wf2, l, D, gT, "wf2", P, resid_evac, nk=FK)

        for l in range(n_layers):
            layer(l)

        if not with_head:
            for j in range(DK):
                nc.sync.dma_start(out[j * P:(j + 1) * P, :], xT[j][:])
        else:
            # ---- LM head: logits[t, v] = x @ wembT ----
            xbf = []
            for j in range(DK):
                xb = acts.tile([P, T], BF16, tag="xbf", bufs=6)
                nc.vector.tensor_copy(xb[:], xT[j][:])
                xbf.append(xb)
            wT3 = wembT.rearrange("(ko ki) v -> ki ko v", ki=P)
            NV = 512
            for vs in range(0, V, NV):
                nn = min(NV, V - vs)
                w_sb = wpool.tile([P, DK, NV], BF16, tag="whead", bufs=2)
                nc.sync.dma_start(w_sb[:, :, :nn], wT3[:, :, vs:vs + nn])
                for tch in range(TCH):
                    ps = ps_gemm.tile([P, NV], FP32, tag="g")
                    for k in range(DK):
                        nc.tensor.matmul(
                            ps[:, :nn], xbf[k][:, tch * P:(tch + 1) * P],
                            w_sb[:, k, :nn], start=(k == 0), stop=(k == DK - 1))
                    o = acts.tile([P, NV], FP32, tag="o_head", bufs=3)
                    if tch % 2 == 0:
                        nc.vector.tensor_copy(o[:, :nn], ps[:, :nn])
                    else:
                        nc.scalar.copy(o[:, :nn], ps[:, :nn])
                    nc.sync.dma_start(out[tch * P:(tch + 1) * P, vs:vs + nn], o[:, :nn])

    nc.compile()
    return nc


# ---------------------------------------------------------------------------
# host side
# ---------------------------------------------------------------------------

B = 8
NCORES = 8


def _np_layer_norm(x, g, b, eps=1e-5):
    m = x.mean(-1, keepdims=True)
    v = x.var(-1, keepdims=True)
    return (x - m) / np.sqrt(v + eps) * g + b


def _prep_in_maps(inputs):
    ids = np.asarray(inputs["input_ids"]).astype(np.int64)
    tt = np.asarray(inputs["token_type_ids"]).astype(np.int64)
    x0 = (np.asarray(inputs["word_emb"], np.float32)[ids]
          + np.asarray(inputs["pos_emb"], np.float32)[None, :ids.shape[1], :]
          + np.asarray(inputs["type_emb"], np.float32)[tt])
    x0 = _np_layer_norm(x0, np.asarray(inputs["emb_ln_g"], np.float32),
                        np.asarray(inputs["emb_ln_b"], np.float32))
    mask = np.asarray(inputs["attention_mask"], np.float32)

    wqkv = np.asarray(inputs["wqkv"], np.float32)
    wfc_in = np.asarray(inputs["wfc"], np.float32)
    ln1_g = np.asarray(inputs["ln1_g"], np.float32)
    ln2_g = np.asarray(inputs["ln2_g"], np.float32)
    for name in ("bqkv", "bproj", "bfc", "bfc2", "ln1_b", "ln2_b"):
        assert np.abs(np.asarray(inputs[name])).max() == 0.0, (
            f"{name} is nonzero; this kernel folds only zero biases")
    wq_eff = wqkv * ln1_g[:, :, None]
    wf_eff = wfc_in * ln2_g[:, :, None]
    packed = dict(
        wqk=np.ascontiguousarray(wq_eff[:, :, :2 * D]).astype(NP_BF16),
        wv=np.ascontiguousarray(wq_eff[:, :, 2 * D:]).astype(NP_BF16),
        wpr=np.asarray(inputs["wproj"], np.float32).astype(NP_BF16),
        wfc=wf_eff.astype(NP_BF16),
        wf2=np.asarray(inputs["wfc2"], np.float32).astype(NP_BF16),
        wembT=np.ascontiguousarray(
            np.asarray(inputs["word_emb"], np.float32).T).astype(NP_BF16),
    )
    in_maps = []
    for b in range(B):
        bias = -10000.0 * (1.0 - mask[b])
        m = dict(packed)
        m["x0T"] = np.ascontiguousarray(x0[b].T).astype(np.float32)
        m["mb"] = np.ascontiguousarray(bias.reshape(TCH, P).T).astype(np.float32)
        in_maps.append(m)
    return in_maps


_NC_CACHE = {}


def get_nc():
    if "nc" not in _NC_CACHE:
        _NC_CACHE["nc"] = build(n_layers=12, with_head=True)
    return _NC_CACHE["nc"]


def kernel(**inputs) -> np.ndarray:
    nc = get_nc()
    in_maps = _prep_in_maps(inputs)
    res = bass_utils.run_bass_kernel_spmd(nc, in_maps, core_ids=list(range(NCORES)))
    return np.stack([res.results[b]["out"] for b in range(B)]).astype(np.float32)



# revision 12
# speedup vs baseline: 1.0779x; 1.0779x over previous
"""BERT-base (12-layer, B=8, T=512, D=768) forward + tied-embedding LM head
on 8 Trainium2 NeuronCores.

Sharding: data-parallel over the batch dimension -- core b computes batch
element b end-to-end (no collectives).

v2 vs the original data-parallel kernel:
- LayerNorm mean-subtraction is folded into the weights host-side (columns of
  every LN-consuming weight are centered, so GEMM(x) == GEMM(x - mean)); the
  rstd scale is applied at PSUM-evacuation time (LN1) or via pre-scaled h2
  tiles (LN2). This removes the LN -> GEMM serialization on the PE.
- Softmax denominators ride along as a 65th AV column (as before) but are
  gathered into one [H, T] tile with tiny SBUF->SBUF DMAs and inverted with a
  single DVE reciprocal_approx_fast -- no per-head Ln/Exp and no per-head DRAM
  round-trips.
- Activation-table thrash fixed: square/ln/exp all resolve to the combined
  natural_log_exp_and_others set (via a compile-time patch of
  get_activation_tables), so only the per-layer gelu switch remains.
- Squares for the variance are computed on the (idle) Vector engine in bf16.
- Weight DMAs are batched into large slabs (fc2: 24 DMAs/layer, not 144).
- Logits are produced in bf16 (host casts to fp32), halving the output DMA.
"""

from contextlib import ExitStack

import numpy as np
import ml_dtypes

import concourse.bass as bass
import concourse.bacc as bacc
import concourse.mybir as mybir
import concourse.tile as tile
from concourse import bass_utils
from concourse._compat import get_trn_type

NP_BF16 = ml_dtypes.bfloat16

FP32 = mybir.dt.float32
BF16 = mybir.dt.bfloat16
AF = mybir.ActivationFunctionType
OP = mybir.AluOpType

P = 128
T = 512
D = 768
H = 12
HD = 64
DF = 3072
V = 30522
DK = D // P      # 6 contraction chunks over D
TCH = T // P     # 4 token chunks
FK = DF // P     # 24 contraction chunks over DF
SCALE = 0.125    # 1/sqrt(64)
EPS = 1e-5


def _patch_act_tables():
    """Make Exp/Ln/Square resolve only to natural_log_exp_and_others (and
    Gelu_apprx_tanh/Tanh only to its set) so the ACT-table-load pass never
    ping-pongs between exp_and_others / natural_log. Indices are preserved
    (sets are filtered, not reordered), so walrus' id mapping is unaffected.
    Returns a restore function."""
    import concourse.hw_specs as hw_specs
    import concourse.bacc as bacc_mod

    orig = hw_specs.get_activation_tables
    combined = {"natural_log_exp_and_others", "gelu_apprx_tanh_and_others"}
    pin = {AF.Exp, AF.Ln, AF.Square}

    def patched(arch):
        t = orig(arch)
        out = {}
        for name, funcs in t.items():
            out[name] = set(funcs) if name in combined else set(funcs) - pin
        return out

    hw_specs.get_activation_tables = patched
    bacc_mod.get_activation_tables = patched

    def restore():
        hw_specs.get_activation_tables = orig
        bacc_mod.get_activation_tables = orig

    return restore


def build(n_layers=12, with_head=True, debug_taps=()):
    nc = bacc.Bacc(get_trn_type() or "TRN2", target_bir_lowering=False, debug=False)

    x0T = nc.dram_tensor("x0T", [D, T], FP32, kind="ExternalInput")
    mb = nc.dram_tensor("mb", [P, TCH], FP32, kind="ExternalInput")
    selin = nc.dram_tensor("selin", [H, H * HD], BF16, kind="ExternalInput")
    L = max(n_layers, 1)
    wqk = nc.dram_tensor("wqk", [L, D, 2 * D], BF16, kind="ExternalInput")
    wv = nc.dram_tensor("wv", [L, D, D], BF16, kind="ExternalInput")
    wpr = nc.dram_tensor("wpr", [L, D, D], BF16, kind="ExternalInput")
    wfc = nc.dram_tensor("wfc", [L, D, DF], BF16, kind="ExternalInput")
    wf2 = nc.dram_tensor("wf2", [L, DF, D], BF16, kind="ExternalInput")
    if with_head:
        wembT = nc.dram_tensor("wembT", [D, V], BF16, kind="ExternalInput")
        out = nc.dram_tensor("out", [T, V], BF16, kind="ExternalOutput")
    else:
        out = nc.dram_tensor("out", [D, T], FP32, kind="ExternalOutput")

    tap_specs = {
        "h1": ([D, T], BF16), "qkT": ([2 * D, T], BF16),
        "v65": ([TCH * P, H * (HD + 1)], BF16), "p0": ([TCH * P, T], BF16),
        "yT": ([D, T], BF16), "xattn": ([D, T], FP32), "h2": ([D, T], BF16),
        "gT": ([DF, T], BF16), "xfinal": ([D, T], FP32),
        "den": ([H, T], FP32), "inv": ([H, T], BF16),
    }
    taps = {}
    for name in debug_taps:
        shape, dt = tap_specs[name]
        taps[name] = nc.dram_tensor(f"tap_{name}", shape, dt, kind="ExternalOutput")

    with tile.TileContext(nc) as tc, ExitStack() as ctx:
        consts = ctx.enter_context(tc.tile_pool(name="consts", bufs=1))
        resid = ctx.enter_context(tc.tile_pool(name="resid", bufs=1))
        acts = ctx.enter_context(tc.tile_pool(name="acts", bufs=2))
        wpool = ctx.enter_context(tc.tile_pool(name="wpool", bufs=1))
        small = ctx.enter_context(tc.tile_pool(name="small", bufs=1))
        ps_stat = ctx.enter_context(tc.tile_pool(name="ps_stat", bufs=2, space="PSUM"))
        ps_gemm = ctx.enter_context(tc.tile_pool(name="ps_gemm", bufs=4, space="PSUM"))
        ps_av = ctx.enter_context(tc.tile_pool(name="ps_av", bufs=2, space="PSUM"))
        dscratch = ctx.enter_context(tc.tile_pool(name="dscratch", bufs=2, space="DRAM"))

        czero = consts.tile([P, 1], FP32, tag="czero")
        nc.vector.memset(czero[:], 0.0)
        ceps = consts.tile([P, 1], FP32, tag="ceps")
        nc.vector.memset(ceps[:], EPS)
        nc.const_aps.aps[(FP32, 0.0)] = czero[:]
        nc.const_aps.aps[(FP32, EPS)] = ceps[:]

        ones_bf = consts.tile([P, 1], BF16, tag="ones_bf")
        nc.vector.memset(ones_bf[:], 1.0)
        ones_row = consts.tile([1, P], BF16, tag="ones_row")
        nc.vector.memset(ones_row[:], 1.0)
        mb_sb = consts.tile([P, TCH], FP32, tag="mb_sb")
        nc.sync.dma_start(mb_sb[:], mb[:])
        # sel_sb[:, h*HD:(h+1)*HD] is [H, HD] with row h all-ones -- broadcasts
        # inv_bf[h] to partitions 0..63 via a K=H matmul (DVE lanes are
        # partition-locked, so every yT multiply wants operands at 0..63).
        # Loaded from DRAM: engine writes can't start at unaligned partitions.
        sel_sb = consts.tile([H, H * HD], BF16, tag="sel")
        nc.sync.dma_start(sel_sb[:], selin[:])
        sel1 = [sel_sb[:, h * HD:(h + 1) * HD] for h in range(H)]

        # residual stream (fp32 master + bf16 shadow)
        xT, xbf = [], []
        for j in range(DK):
            t = resid.tile([P, T], FP32, tag=f"x{j}")
            nc.sync.dma_start(t[:], x0T[j * P:(j + 1) * P, :])
            xT.append(t)
            b = resid.tile([P, T], BF16, tag=f"xb{j}")
            nc.vector.tensor_copy(b[:], t[:])
            xbf.append(b)

        def ln_stats(tag, want_transpose=False):
            """LN stats on xbf -> (rstd_row fp32 [1,T], rstd_b bf16 [P,T] sbuf,
            rstd_T fp32 [P,TCH] or None)."""
            sum_ps = ps_stat.tile([1, T], FP32, tag="stat", name=f"sum_{tag}")
            ssq_ps = ps_stat.tile([1, T], FP32, tag="stat", name=f"ssq_{tag}")
            for j in range(DK):
                sq = acts.tile([P, T], BF16, tag="sq", bufs=3)
                nc.vector.tensor_mul(sq[:], xbf[j][:], xbf[j][:])
                nc.tensor.matmul(sum_ps[:], ones_bf[:], xbf[j][:],
                                 start=(j == 0), stop=(j == DK - 1))
                nc.tensor.matmul(ssq_ps[:], ones_bf[:], sq[:],
                                 start=(j == 0), stop=(j == DK - 1))
            # mean itself is never needed (centered weights absorb it); only
            # m^2 enters the variance
            nm = small.tile([1, T], FP32, tag="stA")
            nc.vector.tensor_scalar_mul(nm[:], sum_ps[:], 1.0 / D)
            msq = small.tile([1, T], FP32, tag="stB")
            nc.vector.tensor_mul(msq[:], nm[:], nm[:])
            var = small.tile([1, T], FP32, tag="stA")
            nc.vector.scalar_tensor_tensor(
                out=var[:], in0=ssq_ps[:], scalar=1.0 / D, in1=msq[:],
                op0=OP.mult, op1=OP.subtract)
            lnv = small.tile([1, T], FP32, tag="stB")
            nc.scalar.activation(lnv[:], var[:], AF.Ln, bias=EPS)
            rstd_row = small.tile([1, T], FP32, tag="rstd")
            nc.scalar.activation(rstd_row[:], lnv[:], AF.Exp, scale=-0.5)
            rstd_row_bf = small.tile([1, T], BF16, tag="rstdbf")
            nc.vector.tensor_copy(rstd_row_bf[:], rstd_row[:])
            # broadcast [1,T] -> [P,T] via ones-matmul, cast to bf16 sbuf
            bc_ps = ps_stat.tile([P, T], FP32, tag="stat", name=f"bc_{tag}")
            nc.tensor.matmul(bc_ps[:], ones_row[:], rstd_row_bf[:],
                             start=True, stop=True)
            rstd_b = acts.tile([P, T], BF16, tag=f"rstdb_{tag}", bufs=2)
            nc.vector.tensor_copy(rstd_b[:], bc_ps[:])
            rstd_T = None
            if want_transpose:
                # transpose [1,T] -> [P,TCH] via a DRAM bounce (DMA cannot
                # balance a partition-spreading SBUF->SBUF access pattern)
                rtd = dscratch.tile([1, T], FP32, tag="rtd", bufs=2)
                nc.sync.dma_start(rtd[:], rstd_row[:])
                rstd_T = acts.tile([P, TCH], FP32, tag="rstdT", bufs=2)
                src = bass.AP(tensor=rtd.tensor, offset=rtd.offset,
                              ap=[[1, P], [P, TCH]])
                nc.sync.dma_start(rstd_T[:], src)
            return rstd_row, rstd_b, rstd_T

        def gemm_fm(w3, l, M, rhs_tiles, tag, CG, evac, nk=DK):
            """feature-major GEMM: psum[128,T] = sum_k w3[l, kchunk, mchunk].T
            @ rhs_tiles[k], column groups of CG limit slab residency."""
            for cg0 in range(0, M, CG):
                cgn = min(CG, M - cg0)
                slabs = []
                for k in range(nk):
                    s = wpool.tile([P, CG], BF16, tag=f"{tag}_{k}", bufs=1)
                    nc.sync.dma_start(s[:, :cgn], w3[l, k * P:(k + 1) * P, cg0:cg0 + cgn])
                    slabs.append(s)
                for mi in range(cgn // P):
                    m = (cg0 // P) + mi
                    ps = ps_gemm.tile([P, T], FP32, tag="g")
                    for k in range(nk):
                        nc.tensor.matmul(ps[:], slabs[k][:, mi * P:(mi + 1) * P],
                                         rhs_tiles[k][:],
                                         start=(k == 0), stop=(k == nk - 1))
                    evac(m, ps)

        def dump_tiles(name, tiles, rows=P):
            if name in taps:
                for j, t in enumerate(tiles):
                    nc.sync.dma_start(taps[name][j * rows:(j + 1) * rows, :], t[:])

        def layer(l):
            # ---- LN1 stats (fold: GEMMs run on xbf, scale at evac) ----
            rstd1_row, rstd1_b, rstd1_T = ln_stats("ln1", want_transpose=True)

            # ---- QK gemm on xbf, evac scales by rstd1 -> qkT bf16 ----
            qkT = [None] * (2 * D // P)

            def qk_evac(m, ps):
                qt = acts.tile([P, T], BF16, tag="qkT", bufs=12)
                nc.vector.tensor_mul(qt[:], ps[:], rstd1_b[:])
                qkT[m] = qt
            gemm_fm(wqk, l, 2 * D, xbf, "wqk", D, qk_evac)
            dump_tiles("qkT", qkT)

            # ---- scores + exp, interleaved with the V gemm so the PE queue
            # ---- never blocks long on the (slower) ACT exp stream ----
            v_slabs = []
            for k in range(DK):
                s = wpool.tile([P, D], BF16, tag=f"wv_{k}", bufs=1)
                nc.sync.dma_start(s[:], wv[l, k * P:(k + 1) * P, :])
                v_slabs.append(s)
            v65 = []

            def v_chunk(tch):
                vt = acts.tile([P, H, HD + 1], BF16, tag="v65", bufs=4)
                nc.vector.memset(vt[:, :, HD:HD + 1], 1.0)
                for n0 in range(0, D, T):
                    nn = min(T, D - n0)
                    ps = ps_gemm.tile([P, T], FP32, tag="g")
                    for k in range(DK):
                        nc.tensor.matmul(
                            ps[:, :nn],
                            xbf[k][:, tch * P:(tch + 1) * P],
                            v_slabs[k][:, n0:n0 + nn],
                            start=(k == 0), stop=(k == DK - 1))
                    dst = vt[:, n0 // HD:(n0 + nn) // HD, 0:HD]
                    src = ps[:, :nn].rearrange("p (h d) -> p h d", d=HD)
                    nc.vector.tensor_scalar(
                        out=dst, in0=src, scalar1=rstd1_T[:, tch:tch + 1],
                        scalar2=None, op0=OP.mult)
                v65.append(vt)

            p_tiles = {}
            for h in range(H):
                ht, r = h // 2, h % 2
                qt = qkT[ht]
                kt = qkT[DK + ht]
                rows = slice(r * HD, (r + 1) * HD)
                for kc in range(TCH):
                    s_ps = ps_gemm.tile([P, T], FP32, tag="g")
                    nc.tensor.matmul(s_ps[:], kt[rows, kc * P:(kc + 1) * P],
                                     qt[rows, :], start=True, stop=True)
                    pt = acts.tile([P, T], BF16, tag="p", bufs=32)
                    nc.scalar.activation(pt[:], s_ps[:], AF.Exp,
                                         bias=mb_sb[:, kc:kc + 1], scale=SCALE)
                    p_tiles[(h, kc)] = pt
                if h % 2 == 1 and h // 2 < TCH:
                    v_chunk(h // 2)
            if "p0" in taps:
                for kc in range(TCH):
                    nc.sync.dma_start(taps["p0"][kc * P:(kc + 1) * P, :],
                                      p_tiles[(0, kc)][:])
            if "v65" in taps:
                for j, t in enumerate(v65):
                    nc.sync.dma_start(
                        taps["v65"][j * P:(j + 1) * P, :],
                        t[:].rearrange("p h d -> p (h d)"))

            # ---- AV per head; stage y+den to SBUF bf16; gather dens ----
            den_bf = small.tile([H, T], BF16, tag="den_bf")
            y65 = []
            for h in range(H):
                y_ps = ps_av.tile([HD + 1, T], FP32, tag="av")
                for kc in range(TCH):
                    nc.tensor.matmul(y_ps[:], v65[kc][:, h, :], p_tiles[(h, kc)][:],
                                     start=(kc == 0), stop=(kc == TCH - 1))
                ysb = acts.tile([HD + 1, T], BF16, tag="y65", bufs=12)
                nc.vector.tensor_copy(ysb[:], y_ps[:])
                nc.sync.dma_start(den_bf[h:h + 1, :], ysb[HD:HD + 1, :])
                y65.append(ysb)

            # ---- batched denominators -> inv; per-head broadcast; yT ----
            den_f = small.tile([H, T], FP32, tag="stA")
            nc.vector.tensor_copy(den_f[:], den_bf[:])
            if "den" in taps:
                nc.sync.dma_start(taps["den"][:], den_f[:])
            inv_f = small.tile([H, T], FP32, tag="stB")
            nc.vector.reciprocal_approx_fast(inv_f[:], den_f[:])
            inv_bf = small.tile([H, T], BF16, tag="inv_bf")
            nc.vector.tensor_copy(inv_bf[:], inv_f[:])
            if "inv" in taps:
                nc.sync.dma_start(taps["inv"][:], inv_bf[:])
            yT = [acts.tile([P, T], BF16, tag="yT", bufs=6, name=f"yT{i}")
                  for i in range(DK)]
            for h in range(H):
                ht, r = h // 2, h % 2
                inv_ps = ps_stat.tile([HD, T], FP32, tag="stat", name=f"invb{h}")
                nc.tensor.matmul(inv_ps[:], sel1[h], inv_bf[:],
                                 start=True, stop=True)
                if r == 0:
                    nc.vector.tensor_mul(yT[ht][0:HD, :], y65[h][0:HD, :],
                                         inv_ps[:])
                else:
                    ytmp = acts.tile([HD, T], BF16, tag="ytmp", bufs=2)
                    nc.vector.tensor_mul(ytmp[:], y65[h][0:HD, :], inv_ps[:])
                    nc.sync.dma_start(yT[ht][HD:P, :], ytmp[:])
            dump_tiles("yT", yT)

            # ---- proj gemm + residual (+ refresh bf16 shadow) ----
            def resid_evac(m, ps):
                nc.vector.tensor_add(xT[m][:], xT[m][:], ps[:])
                nc.vector.tensor_copy(xbf[m][:], xT[m][:])
            gemm_fm(wpr, l, D, yT, "wpr", D, resid_evac)
            dump_tiles("xattn", xT)

            # ---- LN2: pre-scaled h2 = xbf * rstd2_b (mean folded in wfc) ----
            rstd2_row, rstd2_b, _ = ln_stats("ln2")
            h2 = []
            for j in range(DK):
                t = acts.tile([P, T], BF16, tag="h2", bufs=6)
                nc.vector.tensor_mul(t[:], xbf[j][:], rstd2_b[:])
                h2.append(t)
            dump_tiles("h2", h2)

            # ---- fc1 gemm + gelu ----
            gT = [None] * FK

            def gelu_evac(m, ps):
                g = acts.tile([P, T], BF16, tag="gT", bufs=24)
                nc.scalar.activation(g[:], ps[:], AF.Gelu_apprx_tanh)
                gT[m] = g
            gemm_fm(wfc, l, DF, h2, "wfc", D, gelu_evac)
            dump_tiles("gT", gT)

            # ---- fc2 gemm + residual ----
            gemm_fm(wf2, l, D, gT, "wf2", 384, resid_evac, nk=FK)

        for l in range(n_layers):
            layer(l)
        dump_tiles("xfinal", xT)

        if not with_head:
            for j in range(DK):
                nc.sync.dma_start(out[j * P:(j + 1) * P, :], xT[j][:])
        else:
            # ---- LM head: logits[t, v] = x @ wembT (bf16 out) ----
            wT3 = wembT.rearrange("(ko ki) v -> ki ko v", ki=P)
            NV = 512
            for vs in range(0, V, NV):
                nn = min(NV, V - vs)
                w_sb = wpool.tile([P, DK, NV], BF16, tag="whead", bufs=2)
                nc.sync.dma_start(w_sb[:, :, :nn], wT3[:, :, vs:vs + nn])
                for tch in range(TCH):
                    ps = ps_gemm.tile([P, NV], FP32, tag="g")
                    for k in range(DK):
                        nc.tensor.matmul(
                            ps[:, :nn], xbf[k][:, tch * P:(tch + 1) * P],
                            w_sb[:, k, :nn], start=(k == 0), stop=(k == DK - 1))
                    o = acts.tile([P, NV], BF16, tag="o_head", bufs=4)
                    if tch % 2 == 0:
                        nc.vector.tensor_copy(o[:, :nn], ps[:, :nn])
                    else:
                        nc.scalar.copy(o[:, :nn], ps[:, :nn])
                    nc.sync.dma_start(out[tch * P:(tch + 1) * P, vs:vs + nn], o[:, :nn])

    restore = _patch_act_tables()
    try:
        nc.compile()
    finally:
        restore()
    return nc


# ---------------------------------------------------------------------------
# host side
# ---------------------------------------------------------------------------

B = 8
NCORES = 8


def _np_layer_norm(x, g, b, eps=1e-5):
    m = x.mean(-1, keepdims=True)
    v = x.var(-1, keepdims=True)
    return (x - m) / np.sqrt(v + eps) * g + b


def _prep_in_maps(inputs):
    ids = np.asarray(inputs["input_ids"]).astype(np.int64)
    tt = np.asarray(inputs["token_type_ids"]).astype(np.int64)
    x0 = (np.asarray(inputs["word_emb"], np.float32)[ids]
          + np.asarray(inputs["pos_emb"], np.float32)[None, :ids.shape[1], :]
          + np.asarray(inputs["type_emb"], np.float32)[tt])
    x0 = _np_layer_norm(x0, np.asarray(inputs["emb_ln_g"], np.float32),
                        np.asarray(inputs["emb_ln_b"], np.float32))
    mask = np.asarray(inputs["attention_mask"], np.float32)

    wqkv = np.asarray(inputs["wqkv"], np.float32)
    wfc_in = np.asarray(inputs["wfc"], np.float32)
    ln1_g = np.asarray(inputs["ln1_g"], np.float32)
    ln2_g = np.asarray(inputs["ln2_g"], np.float32)
    for name in ("bqkv", "bproj", "bfc", "bfc2", "ln1_b", "ln2_b"):
        assert np.abs(np.asarray(inputs[name])).max() == 0.0, (
            f"{name} is nonzero; this kernel folds only zero biases")
    # fold LN gain into the consuming weights, then center the columns so the
    # GEMM of (uncentered) x equals the GEMM of (x - mean): sum_d W~[d,o] = 0
    wq_eff = wqkv * ln1_g[:, :, None]
    wq_eff = wq_eff - wq_eff.mean(axis=1, keepdims=True)
    wf_eff = wfc_in * ln2_g[:, :, None]
    wf_eff = wf_eff - wf_eff.mean(axis=1, keepdims=True)
    sel = np.zeros((H, H, HD), np.float32)
    for h in range(H):
        sel[h, h, :] = 1.0
    packed = dict(
        selin=np.ascontiguousarray(sel.reshape(H, H * HD)).astype(NP_BF16),
        wqk=np.ascontiguousarray(wq_eff[:, :, :2 * D]).astype(NP_BF16),
        wv=np.ascontiguousarray(wq_eff[:, :, 2 * D:]).astype(NP_BF16),
        wpr=np.asarray(inputs["wproj"], np.float32).astype(NP_BF16),
        wfc=wf_eff.astype(NP_BF16),
        wf2=np.asarray(inputs["wfc2"], np.float32).astype(NP_BF16),
        wembT=np.ascontiguousarray(
            np.asarray(inputs["word_emb"], np.float32).T).astype(NP_BF16),
    )
    in_maps = []
    for b in range(B):
        bias = -10000.0 * (1.0 - mask[b])
        m = dict(packed)
        m["x0T"] = np.ascontiguousarray(x0[b].T).astype(np.float32)
        m["mb"] = np.ascontiguousarray(bias.reshape(TCH, P).T).astype(np.float32)
        in_maps.append(m)
    return in_maps


_NC_CACHE = {}


def get_nc():
    if "nc" not in _NC_CACHE:
        _NC_CACHE["nc"] = build(n_layers=12, with_head=True)
    return _NC_CACHE["nc"]


def kernel(**inputs) -> np.ndarray:
    nc = get_nc()
    in_maps = _prep_in_maps(inputs)
    res = bass_utils.run_bass_kernel_spmd(nc, in_maps, core_ids=list(range(NCORES)))
    return np.stack([res.results[b]["out"] for b in range(B)]).astype(np.float32)


# revision 17
# speedup vs baseline: 2.1632x; 2.0068x over previous
"""BERT-base (12-layer, B=8, T=512, D=768) forward + tied-embedding LM head
on 8 Trainium2 NeuronCores.

Sharding: data-parallel over the batch dimension -- core b computes batch
element b end-to-end (no collectives).

v2 vs the original data-parallel kernel:
- LayerNorm mean-subtraction is folded into the weights host-side (columns of
  every LN-consuming weight are centered, so GEMM(x) == GEMM(x - mean)); the
  rstd scale is applied at PSUM-evacuation time (LN1) or via pre-scaled h2
  tiles (LN2). This removes the LN -> GEMM serialization on the PE.
- Softmax denominators ride along as a 65th AV column (as before) but are
  gathered into one [H, T] tile with tiny SBUF->SBUF DMAs and inverted with a
  single DVE reciprocal_approx_fast -- no per-head Ln/Exp and no per-head DRAM
  round-trips.
- Activation-table thrash fixed: square/ln/exp all resolve to the combined
  natural_log_exp_and_others set (via a compile-time patch of
  get_activation_tables), so only the per-layer gelu switch remains.
- Squares for the variance are computed on the (idle) Vector engine in bf16.
- Weight DMAs are batched into large slabs (fc2: 24 DMAs/layer, not 144).
- Logits are produced in bf16 (host casts to fp32), halving the output DMA.
"""

from contextlib import ExitStack

import numpy as np
import ml_dtypes

import concourse.bass as bass
import concourse.bacc as bacc
import concourse.mybir as mybir
import concourse.tile as tile
from concourse import bass_utils
from concourse._compat import get_trn_type

NP_BF16 = ml_dtypes.bfloat16

FP32 = mybir.dt.float32
BF16 = mybir.dt.bfloat16
AF = mybir.ActivationFunctionType
OP = mybir.AluOpType

P = 128
T = 512
D = 768
H = 12
HD = 64
DF = 3072
V = 30522
DK = D // P      # 6 contraction chunks over D
TCH = T // P     # 4 token chunks
FK = DF // P     # 24 contraction chunks over DF
SCALE = 0.125    # 1/sqrt(64)
EPS = 1e-5


def _patch_act_tables():
    """Make Exp/Ln/Square resolve only to natural_log_exp_and_others (and
    Gelu_apprx_tanh/Tanh only to its set) so the ACT-table-load pass never
    ping-pongs between exp_and_others / natural_log. Indices are preserved
    (sets are filtered, not reordered), so walrus' id mapping is unaffected.
    Returns a restore function."""
    import concourse.hw_specs as hw_specs
    import concourse.bacc as bacc_mod

    orig = hw_specs.get_activation_tables
    combined = {"natural_log_exp_and_others", "gelu_apprx_tanh_and_others"}
    pin = {AF.Exp, AF.Ln, AF.Square}

    def patched(arch):
        t = orig(arch)
        out = {}
        for name, funcs in t.items():
            out[name] = set(funcs) if name in combined else set(funcs) - pin
        return out

    hw_specs.get_activation_tables = patched
    bacc_mod.get_activation_tables = patched

    def restore():
        hw_specs.get_activation_tables = orig
        bacc_mod.get_activation_tables = orig

    return restore


def build(n_layers=12, with_head=True, debug_taps=()):
    nc = bacc.Bacc(get_trn_type() or "TRN2", target_bir_lowering=False, debug=False)

    x0T = nc.dram_tensor("x0T", [D, T], FP32, kind="ExternalInput")
    mb = nc.dram_tensor("mb", [P, TCH], FP32, kind="ExternalInput")
    selin = nc.dram_tensor("selin", [H, H * HD], BF16, kind="ExternalInput")
    L = max(n_layers, 1)
    wqk = nc.dram_tensor("wqk", [L, D, 2 * D], BF16, kind="ExternalInput")
    wv = nc.dram_tensor("wv", [L, D, D], BF16, kind="ExternalInput")
    wpr = nc.dram_tensor("wpr", [L, D, D], BF16, kind="ExternalInput")
    wfc = nc.dram_tensor("wfc", [L, D, DF], BF16, kind="ExternalInput")
    wf2 = nc.dram_tensor("wf2", [L, DF, D], BF16, kind="ExternalInput")
    if with_head:
        wembT = nc.dram_tensor("wembT", [D, V], BF16, kind="ExternalInput")
        out = nc.dram_tensor("out", [T, V], BF16, kind="ExternalOutput")
    else:
        out = nc.dram_tensor("out", [D, T], FP32, kind="ExternalOutput")

    tap_specs = {
        "h1": ([D, T], BF16), "qkT": ([2 * D, T], BF16),
        "v65": ([TCH * P, H * (HD + 1)], BF16), "p0": ([TCH * P, T], BF16),
        "yT": ([D, T], BF16), "xattn": ([D, T], FP32), "h2": ([D, T], BF16),
        "gT": ([DF, T], BF16), "xfinal": ([D, T], FP32),
        "den": ([H, T], FP32), "inv": ([H, T], BF16),
    }
    taps = {}
    for name in debug_taps:
        shape, dt = tap_specs[name]
        taps[name] = nc.dram_tensor(f"tap_{name}", shape, dt, kind="ExternalOutput")

    with tile.TileContext(nc) as tc, ExitStack() as ctx:
        consts = ctx.enter_context(tc.tile_pool(name="consts", bufs=1))
        resid = ctx.enter_context(tc.tile_pool(name="resid", bufs=1))
        acts = ctx.enter_context(tc.tile_pool(name="acts", bufs=2))
        wpool = ctx.enter_context(tc.tile_pool(name="wpool", bufs=1))
        small = ctx.enter_context(tc.tile_pool(name="small", bufs=1))
        ps_stat = ctx.enter_context(tc.tile_pool(name="ps_stat", bufs=2, space="PSUM"))
        ps_gemm = ctx.enter_context(tc.tile_pool(name="ps_gemm", bufs=4, space="PSUM"))
        ps_av = ctx.enter_context(tc.tile_pool(name="ps_av", bufs=2, space="PSUM"))
        dscratch = ctx.enter_context(tc.tile_pool(name="dscratch", bufs=2, space="DRAM"))

        czero = consts.tile([P, 1], FP32, tag="czero")
        nc.vector.memset(czero[:], 0.0)
        ceps = consts.tile([P, 1], FP32, tag="ceps")
        nc.vector.memset(ceps[:], EPS)
        nc.const_aps.aps[(FP32, 0.0)] = czero[:]
        nc.const_aps.aps[(FP32, EPS)] = ceps[:]

        ones_bf = consts.tile([P, 1], BF16, tag="ones_bf")
        nc.vector.memset(ones_bf[:], 1.0)
        ones_row = consts.tile([1, P], BF16, tag="ones_row")
        nc.vector.memset(ones_row[:], 1.0)
        mb_sb = consts.tile([P, TCH], FP32, tag="mb_sb")
        nc.sync.dma_start(mb_sb[:], mb[:])
        # sel_sb[:, h*HD:(h+1)*HD] is [H, HD] with row h all-ones -- broadcasts
        # inv_bf[h] to partitions 0..63 via a K=H matmul (DVE lanes are
        # partition-locked, so every yT multiply wants operands at 0..63).
        # Loaded from DRAM: engine writes can't start at unaligned partitions.
        sel_sb = consts.tile([H, H * HD], BF16, tag="sel")
        nc.sync.dma_start(sel_sb[:], selin[:])
        sel1 = [sel_sb[:, h * HD:(h + 1) * HD] for h in range(H)]

        # residual stream (fp32 master + bf16 shadow)
        xT, xbf = [], []
        for j in range(DK):
            t = resid.tile([P, T], FP32, tag=f"x{j}")
            nc.sync.dma_start(t[:], x0T[j * P:(j + 1) * P, :])
            xT.append(t)
            b = resid.tile([P, T], BF16, tag=f"xb{j}")
            nc.vector.tensor_copy(b[:], t[:])
            xbf.append(b)

        def ln_stats(tag, want_transpose=False):
            """LN stats on xbf -> (rstd_row fp32 [1,T], rstd_b bf16 [P,T] sbuf,
            rstd_T fp32 [P,TCH] or None)."""
            sum_ps = ps_stat.tile([1, T], FP32, tag="stat", name=f"sum_{tag}")
            ssq_ps = ps_stat.tile([1, T], FP32, tag="stat", name=f"ssq_{tag}")
            for j in range(DK):
                sq = acts.tile([P, T], BF16, tag="sq", bufs=2)
                nc.vector.tensor_mul(sq[:], xbf[j][:], xbf[j][:])
                nc.tensor.matmul(sum_ps[:], ones_bf[:], xbf[j][:],
                                 start=(j == 0), stop=(j == DK - 1))
                nc.tensor.matmul(ssq_ps[:], ones_bf[:], sq[:],
                                 start=(j == 0), stop=(j == DK - 1))
            # mean itself is never needed (centered weights absorb it); only
            # m^2 enters the variance
            nm = small.tile([1, T], FP32, tag="stA")
            nc.vector.tensor_scalar_mul(nm[:], sum_ps[:], 1.0 / D)
            msq = small.tile([1, T], FP32, tag="stB")
            nc.vector.tensor_mul(msq[:], nm[:], nm[:])
            var = small.tile([1, T], FP32, tag="stA")
            nc.vector.scalar_tensor_tensor(
                out=var[:], in0=ssq_ps[:], scalar=1.0 / D, in1=msq[:],
                op0=OP.mult, op1=OP.subtract)
            lnv = small.tile([1, T], FP32, tag="stB")
            nc.scalar.activation(lnv[:], var[:], AF.Ln, bias=EPS)
            rstd_row = small.tile([1, T], FP32, tag="rstd")
            nc.scalar.activation(rstd_row[:], lnv[:], AF.Exp, scale=-0.5)
            rstd_row_bf = small.tile([1, T], BF16, tag="rstdbf")
            nc.vector.tensor_copy(rstd_row_bf[:], rstd_row[:])
            # broadcast [1,T] -> [P,T] via ones-matmul, cast to bf16 sbuf
            bc_ps = ps_stat.tile([P, T], FP32, tag="stat", name=f"bc_{tag}")
            nc.tensor.matmul(bc_ps[:], ones_row[:], rstd_row_bf[:],
                             start=True, stop=True)
            rstd_b = acts.tile([P, T], BF16, tag=f"rstdb_{tag}", bufs=1)
            nc.vector.tensor_copy(rstd_b[:], bc_ps[:])
            rstd_T = None
            if want_transpose:
                # transpose [1,T] -> [P,TCH] via a DRAM bounce (DMA cannot
                # balance a partition-spreading SBUF->SBUF access pattern)
                rtd = dscratch.tile([1, T], FP32, tag="rtd", bufs=2)
                nc.sync.dma_start(rtd[:], rstd_row[:])
                rstd_T = acts.tile([P, TCH], FP32, tag="rstdT", bufs=2)
                src = bass.AP(tensor=rtd.tensor, offset=rtd.offset,
                              ap=[[1, P], [P, TCH]])
                nc.sync.dma_start(rstd_T[:], src)
            return rstd_row, rstd_b, rstd_T

        def gemm_fm(w3, l, M, rhs_tiles, tag, CG, evac, nk=DK, wbufs=2):
            """feature-major GEMM: psum[128,T] = sum_k w3[l, kchunk, mchunk].T
            @ rhs_tiles[k]. One DMA per column group loads all nk k-slabs."""
            for cg0 in range(0, M, CG):
                cgn = min(CG, M - cg0)
                ws = wpool.tile([P, nk, CG], BF16, tag=tag, bufs=wbufs)
                nc.sync.dma_start(
                    ws[:, :, :cgn],
                    w3[l, 0:nk * P, cg0:cg0 + cgn].rearrange(
                        "(k p) n -> p k n", p=P))
                for mi in range(cgn // P):
                    m = (cg0 // P) + mi
                    ps = ps_gemm.tile([P, T], FP32, tag="g")
                    for k in range(nk):
                        nc.tensor.matmul(ps[:], ws[:, k, mi * P:(mi + 1) * P],
                                         rhs_tiles[k][:],
                                         start=(k == 0), stop=(k == nk - 1))
                    evac(m, ps)

        def dump_tiles(name, tiles, rows=P):
            if name in taps:
                for j, t in enumerate(tiles):
                    nc.sync.dma_start(taps[name][j * rows:(j + 1) * rows, :], t[:])

        def layer(l):
            # ---- LN1 stats (fold: GEMMs run on xbf, scale at evac) ----
            rstd1_row, rstd1_b, rstd1_T = ln_stats("ln1", want_transpose=True)

            # ---- QK gemm on xbf, evac scales by rstd1 -> qkT bf16 ----
            qkT = [None] * (2 * D // P)

            def qk_evac(m, ps):
                qt = acts.tile([P, T], BF16, tag="qkT", bufs=12)
                nc.vector.tensor_mul(qt[:], ps[:], rstd1_b[:])
                qkT[m] = qt
            gemm_fm(wqk, l, 2 * D, xbf, "wqk", D, qk_evac)
            dump_tiles("qkT", qkT)

            # ---- scores + exp, interleaved with the V gemm so the PE queue
            # ---- never blocks long on the (slower) ACT exp stream ----
            wv_all = wpool.tile([P, DK, D], BF16, tag="wv", bufs=1)
            nc.sync.dma_start(
                wv_all[:], wv[l, 0:D, 0:D].rearrange("(k p) n -> p k n", p=P))
            v_slabs = [wv_all[:, k, :] for k in range(DK)]
            v65 = []

            def v_chunk(tch):
                vt = acts.tile([P, H, HD + 1], BF16, tag="v65", bufs=4)
                nc.vector.memset(vt[:, :, HD:HD + 1], 1.0)
                for n0 in range(0, D, T):
                    nn = min(T, D - n0)
                    ps = ps_gemm.tile([P, T], FP32, tag="g")
                    for k in range(DK):
                        nc.tensor.matmul(
                            ps[:, :nn],
                            xbf[k][:, tch * P:(tch + 1) * P],
                            v_slabs[k][:, n0:n0 + nn],
                            start=(k == 0), stop=(k == DK - 1))
                    dst = vt[:, n0 // HD:(n0 + nn) // HD, 0:HD]
                    src = ps[:, :nn].rearrange("p (h d) -> p h d", d=HD)
                    nc.vector.tensor_scalar(
                        out=dst, in0=src, scalar1=rstd1_T[:, tch:tch + 1],
                        scalar2=None, op0=OP.mult)
                v65.append(vt)

            p_tiles = {}
            for h in range(H):
                ht, r = h // 2, h % 2
                qt = qkT[2 * ht]
                kt = qkT[2 * ht + 1]
                rows = slice(r * HD, (r + 1) * HD)
                for kc in range(TCH):
                    s_ps = ps_gemm.tile([P, T], FP32, tag="g")
                    nc.tensor.matmul(s_ps[:], kt[rows, kc * P:(kc + 1) * P],
                                     qt[rows, :], start=True, stop=True)
                    pt = acts.tile([P, T], BF16, tag="p", bufs=24)
                    nc.scalar.activation(pt[:], s_ps[:], AF.Exp,
                                         bias=mb_sb[:, kc:kc + 1], scale=SCALE)
                    p_tiles[(h, kc)] = pt
                if h % 2 == 1 and h // 2 < TCH:
                    v_chunk(h // 2)
            if "p0" in taps:
                for kc in range(TCH):
                    nc.sync.dma_start(taps["p0"][kc * P:(kc + 1) * P, :],
                                      p_tiles[(0, kc)][:])
            if "v65" in taps:
                for j, t in enumerate(v65):
                    nc.sync.dma_start(
                        taps["v65"][j * P:(j + 1) * P, :],
                        t[:].rearrange("p h d -> p (h d)"))

            # ---- AV per head; stage y+den to SBUF bf16; gather dens ----
            den_bf = small.tile([H, T], BF16, tag="den_bf")
            y65 = []
            for h in range(H):
                y_ps = ps_av.tile([HD + 1, T], FP32, tag="av")
                for kc in range(TCH):
                    nc.tensor.matmul(y_ps[:], v65[kc][:, h, :], p_tiles[(h, kc)][:],
                                     start=(kc == 0), stop=(kc == TCH - 1))
                ysb = acts.tile([HD + 1, T], BF16, tag="y65", bufs=12)
                nc.vector.tensor_copy(ysb[:], y_ps[:])
                nc.sync.dma_start(den_bf[h:h + 1, :], ysb[HD:HD + 1, :])
                y65.append(ysb)

            # ---- batched denominators -> inv; per-head broadcast; yT ----
            den_f = small.tile([H, T], FP32, tag="stA")
            nc.vector.tensor_copy(den_f[:], den_bf[:])
            if "den" in taps:
                nc.sync.dma_start(taps["den"][:], den_f[:])
            inv_f = small.tile([H, T], FP32, tag="stB")
            nc.vector.reciprocal_approx_fast(inv_f[:], den_f[:])
            inv_bf = small.tile([H, T], BF16, tag="inv_bf")
            nc.vector.tensor_copy(inv_bf[:], inv_f[:])
            if "inv" in taps:
                nc.sync.dma_start(taps["inv"][:], inv_bf[:])
            yT = [acts.tile([P, T], BF16, tag="yT", bufs=6, name=f"yT{i}")
                  for i in range(DK)]
            for h in range(H):
                ht, r = h // 2, h % 2
                inv_ps = ps_stat.tile([HD, T], FP32, tag="stat", name=f"invb{h}")
                nc.tensor.matmul(inv_ps[:], sel1[h], inv_bf[:],
                                 start=True, stop=True)
                if r == 0:
                    nc.vector.tensor_mul(yT[ht][0:HD, :], y65[h][0:HD, :],
                                         inv_ps[:])
                else:
                    ytmp = acts.tile([HD, T], BF16, tag="ytmp", bufs=2)
                    nc.vector.tensor_mul(ytmp[:], y65[h][0:HD, :], inv_ps[:])
                    nc.sync.dma_start(yT[ht][HD:P, :], ytmp[:])
            dump_tiles("yT", yT)

            # ---- proj gemm + residual (+ refresh bf16 shadow) ----
            def resid_evac(m, ps):
                nc.vector.tensor_add(xT[m][:], xT[m][:], ps[:])
                nc.vector.tensor_copy(xbf[m][:], xT[m][:])
            gemm_fm(wpr, l, D, yT, "wpr", D, resid_evac, wbufs=1)
            dump_tiles("xattn", xT)

            # ---- LN2: pre-scaled h2 = xbf * rstd2_b (mean folded in wfc) ----
            rstd2_row, rstd2_b, _ = ln_stats("ln2")
            h2 = []
            for j in range(DK):
                t = acts.tile([P, T], BF16, tag="h2", bufs=6)
                nc.vector.tensor_mul(t[:], xbf[j][:], rstd2_b[:])
                h2.append(t)
            dump_tiles("h2", h2)

            # ---- fc1 gemm + gelu ----
            gT = [None] * FK

            def gelu_evac(m, ps):
                g = acts.tile([P, T], BF16, tag="gT", bufs=24)
                nc.scalar.activation(g[:], ps[:], AF.Gelu_apprx_tanh)
                gT[m] = g
            gemm_fm(wfc, l, DF, h2, "wfc", D, gelu_evac)
            dump_tiles("gT", gT)

            # ---- fc2 gemm + residual ----
            gemm_fm(wf2, l, D, gT, "wf2", 128, resid_evac, nk=FK)

        for l in range(n_layers):
            layer(l)
        dump_tiles("xfinal", xT)

        if not with_head:
            for j in range(DK):
                nc.sync.dma_start(out[j * P:(j + 1) * P, :], xT[j][:])
        else:
            # ---- LM head: logits[t, v] = x @ wembT (bf16 out) ----
            wT3 = wembT.rearrange("(ko ki) v -> ki ko v", ki=P)
            NV = 512
            for vs in range(0, V, NV):
                nn = min(NV, V - vs)
                w_sb = wpool.tile([P, DK, NV], BF16, tag="whead", bufs=2)
                nc.sync.dma_start(w_sb[:, :, :nn], wT3[:, :, vs:vs + nn])
                for tch in range(TCH):
                    ps = ps_gemm.tile([P, NV], FP32, tag="g")
                    for k in range(DK):
                        nc.tensor.matmul(
                            ps[:, :nn], xbf[k][:, tch * P:(tch + 1) * P],
                            w_sb[:, k, :nn], start=(k == 0), stop=(k == DK - 1))
                    o = acts.tile([P, NV], BF16, tag="o_head", bufs=3)
                    if tch % 2 == 0:
                        nc.vector.tensor_copy(o[:, :nn], ps[:, :nn])
                    else:
                        nc.scalar.copy(o[:, :nn], ps[:, :nn])
                    nc.sync.dma_start(out[tch * P:(tch + 1) * P, vs:vs + nn], o[:, :nn])

    restore = _patch_act_tables()
    try:
        nc.compile()
    finally:
        restore()
    return nc


# ---------------------------------------------------------------------------
# host side
# ---------------------------------------------------------------------------

B = 8
NCORES = 8


def _np_layer_norm(x, g, b, eps=1e-5):
    m = x.mean(-1, keepdims=True)
    v = x.var(-1, keepdims=True)
    return (x - m) / np.sqrt(v + eps) * g + b


def _prep_in_maps(inputs):
    ids = np.asarray(inputs["input_ids"]).astype(np.int64)
    tt = np.asarray(inputs["token_type_ids"]).astype(np.int64)
    x0 = (np.asarray(inputs["word_emb"], np.float32)[ids]
          + np.asarray(inputs["pos_emb"], np.float32)[None, :ids.shape[1], :]
          + np.asarray(inputs["type_emb"], np.float32)[tt])
    x0 = _np_layer_norm(x0, np.asarray(inputs["emb_ln_g"], np.float32),
                        np.asarray(inputs["emb_ln_b"], np.float32))
    mask = np.asarray(inputs["attention_mask"], np.float32)

    wqkv = np.asarray(inputs["wqkv"], np.float32)
    wfc_in = np.asarray(inputs["wfc"], np.float32)
    ln1_g = np.asarray(inputs["ln1_g"], np.float32)
    ln2_g = np.asarray(inputs["ln2_g"], np.float32)
    for name in ("bqkv", "bproj", "bfc", "bfc2", "ln1_b", "ln2_b"):
        assert np.abs(np.asarray(inputs[name])).max() == 0.0, (
            f"{name} is nonzero; this kernel folds only zero biases")
    # fold LN gain into the consuming weights, then center the columns so the
    # GEMM of (uncentered) x equals the GEMM of (x - mean): sum_d W~[d,o] = 0
    wq_eff = wqkv * ln1_g[:, :, None]
    wq_eff = wq_eff - wq_eff.mean(axis=1, keepdims=True)
    wf_eff = wfc_in * ln2_g[:, :, None]
    wf_eff = wf_eff - wf_eff.mean(axis=1, keepdims=True)
    sel = np.zeros((H, H, HD), np.float32)
    for h in range(H):
        sel[h, h, :] = 1.0
    wq_part = wq_eff[:, :, :D]
    wk_part = wq_eff[:, :, D:2 * D]
    Lw = wq_eff.shape[0]
    wqk_il = np.empty((Lw, D, 2 * D), np.float32)
    for i in range(D // 128):
        wqk_il[:, :, i * 256:i * 256 + 128] = wq_part[:, :, i * 128:(i + 1) * 128]
        wqk_il[:, :, i * 256 + 128:(i + 1) * 256] = wk_part[:, :, i * 128:(i + 1) * 128]
    packed = dict(
        selin=np.ascontiguousarray(sel.reshape(H, H * HD)).astype(NP_BF16),
        wqk=np.ascontiguousarray(wqk_il).astype(NP_BF16),
        wv=np.ascontiguousarray(wq_eff[:, :, 2 * D:]).astype(NP_BF16),
        wpr=np.asarray(inputs["wproj"], np.float32).astype(NP_BF16),
        wfc=wf_eff.astype(NP_BF16),
        wf2=np.asarray(inputs["wfc2"], np.float32).astype(NP_BF16),
        wembT=np.ascontiguousarray(
            np.asarray(inputs["word_emb"], np.float32).T).astype(NP_BF16),
    )
    in_maps = []
    for b in range(B):
        bias = -10000.0 * (1.0 - mask[b])
        m = dict(packed)
        m["x0T"] = np.ascontiguousarray(x0[b].T).astype(np.float32)
        m["mb"] = np.ascontiguousarray(bias.reshape(TCH, P).T).astype(np.float32)
        in_maps.append(m)
    return in_maps


_NC_CACHE = {}


def get_nc():
    if "nc" not in _NC_CACHE:
        _NC_CACHE["nc"] = build(n_layers=12, with_head=True)
    return _NC_CACHE["nc"]


def kernel(**inputs) -> np.ndarray:
    nc = get_nc()
    in_maps = _prep_in_maps(inputs)
    res = bass_utils.run_bass_kernel_spmd(nc, in_maps, core_ids=list(range(NCORES)))
    return np.stack([res.results[b]["out"] for b in range(B)]).astype(np.float32)
